# revision 40
# baseline (speedup 1.0000x reference)
"""KGATConv GNN message-passing kernel for 8 Trainium2 NeuronCores.

Strategy (dst-node ownership; fp16 staging + on-device AllGather):
  - Core k owns nodes [k*12500, (k+1)*12500).  Host stages only core k's
    own feature rows (fp16, padded to 12544); the full gather table is
    rebuilt on-device with an HBM AllGather over NeuronLink -- host->device
    traffic for nfeat drops 16x vs replicating f32 to all cores.
  - Host sorts edges by dst and buckets per (core, 128-node window), padding
    each window's edge run to whole 128-edge chunks (chunk counts shared
    across cores so all 8 run one SPMD program).  Edge payload ships fp16
    (dst window offsets, weights, W^T) in one array + int32 src in another.
  - Device, per chunk: indirect-DMA gather of 128 fp16 rows (one offset per
    partition); DVE builds A[p,j] = w_p * (dst_p == j) in fp16; PE fp16
    matmul-accumulates h_nb = A^T @ msg in f32 PSUM.  Finalize per window:
    X = nfeat_own * h_nb, X^T via PE transpose, out = X @ W^T on PE (fp16),
    LeakyReLU on ACT, fp16 DMA out.
  - The PJRT executable, donated-zero outputs, and NEFF are all cached at
    module level so warm calls pay only staging + exec + fetch.
"""

import sys

sys.path.insert(0, "/opt/trn_rl_repo")

from contextlib import ExitStack

import numpy as np

import concourse.bass as bass
import concourse.mybir as mybir
import concourse.tile as tile

N_CORES = 8
D = 128
WIN = 128
NPC = 12500
NWIN = (NPC + WIN - 1) // WIN  # 98
PADN = NWIN * WIN  # 12544
GN = N_CORES * PADN  # 100352

_cache = {}
_USE_QUEUES = False


def _split_excess_waits(nc, maxw=1):
    # This walrus build rejects instructions carrying more than one sync
    # wait; move extras onto preceding single-wait NoOps on the same engine.
    for f in nc.m.functions:
        for bb in f.blocks:
            out = []
            for inst in bb.instructions:
                si = inst.sync_info
                waits = list(si.on_wait) if si and si.on_wait else []
                if len(waits) > maxw:
                    extra, keep = waits[:-maxw], waits[-maxw:]
                    for i in range(0, len(extra), maxw):
                        nop = mybir.InstNoOp(
                            name=nc.get_next_instruction_name(), ins=[], outs=[]
                        )
                        nop.engine = inst.engine
                        nop.sync_info = type(si)(
                            on_wait=extra[i : i + maxw], on_update=[]
                        )
                        nc.register_instruction(nop, overwrite=True)
                        out.append(nop)
                    si.on_wait = keep
                out.append(inst)
            bb.instructions[:] = out


def _build_nc(ct, c_list):
    f16 = mybir.dt.float16
    f32 = mybir.dt.float32
    nc = bass.Bass(num_swdge_queues=4)
    assert ct % 2 == 0
    nfsh_d = nc.declare_dram_parameter("nfsh", [PADN, D], f16, isOutput=False)
    # rows 0:128 dst-offset f16, 128:256 w f16, 256:512 the int32 src indices
    # bitcast to f16 (two half-row blocks); W^T at [0:128, ct:ct+128].
    # One param = one host device_put (~100ms saved vs a separate int32 put).
    edgf_d = nc.declare_dram_parameter("edgf", [512, ct + 128], f16, isOutput=False)
    # int8 output with a per-row f32 scale (row absmax / 127) halves the
    # device->host fetch; quantization error is <= 1/127 of the row max.
    # Partition-major layout [p, t*D+d] so the whole thing ships as one DMA
    # (host untransposes).  The f32 scales ride along bitcast into the last
    # NWIN*4 int8 columns: a second ExternalOutput costs ~100ms of dispatch.
    out_d = nc.declare_dram_parameter(
        "out", [128, NWIN * D + NWIN * 4], mybir.dt.int8, isOutput=True
    )
    iota_d = nc.inline_tensor(
        np.tile(np.arange(WIN, dtype=np.float32), (128, 1)), name="iota"
    )
    ident_d = nc.inline_tensor(np.eye(128, dtype=np.float32), name="ident")

    with tile.TileContext(nc) as tc, ExitStack() as ctx:
        dram = ctx.enter_context(tc.tile_pool(name="dram", bufs=1, space="DRAM"))
        agin = dram.tile([PADN, D], f16)
        agout = dram.tile([GN, D], f16, addr_space="Shared")
        const = ctx.enter_context(tc.tile_pool(name="const", bufs=1))
        gp = ctx.enter_context(tc.tile_pool(name="gp", bufs=10))
        ap_pool = ctx.enter_context(tc.tile_pool(name="ap", bufs=4))
        wk = ctx.enter_context(tc.tile_pool(name="wk", bufs=3))
        ps = ctx.enter_context(tc.tile_pool(name="ps", bufs=2, space="PSUM"))

        # kick off the gather-table rebuild first; const loads overlap it
        nc.sync.dma_start(out=agin[:], in_=nfsh_d[:])
        nc.gpsimd.collective_compute(
            "AllGather",
            mybir.AluOpType.bypass,
            replica_groups=[list(range(N_CORES))],
            ins=[agin.opt()],
            outs=[agout.opt()],
        )

        srci_sb = const.tile([128, ct], mybir.dt.int32)
        nc.sync.dma_start(
            out=srci_sb[:, 0 : ct // 2].bitcast(f16), in_=edgf_d[256:384, 0:ct]
        )
        nc.sync.dma_start(
            out=srci_sb[:, ct // 2 : ct].bitcast(f16), in_=edgf_d[384:512, 0:ct]
        )
        dst_sb16 = const.tile([128, ct], f16)
        nc.sync.dma_start(out=dst_sb16[:], in_=edgf_d[0:128, 0:ct])
        w_sb16 = const.tile([128, ct], f16)
        nc.sync.dma_start(out=w_sb16[:], in_=edgf_d[128:256, 0:ct])
        # DVE scalar operands must be f32; widen once on device
        dst_sb = const.tile([128, ct], f32)
        nc.scalar.activation(
            out=dst_sb[:], in_=dst_sb16[:], func=mybir.ActivationFunctionType.Copy
        )
        w_sb = const.tile([128, ct], f32)
        nc.scalar.activation(
            out=w_sb[:], in_=w_sb16[:], func=mybir.ActivationFunctionType.Copy
        )
        wt_sb = const.tile([128, 128], f16)
        nc.sync.dma_start(out=wt_sb[:], in_=edgf_d[0:128, ct : ct + 128])
        iota_sb = const.tile([128, WIN], f32)
        nc.sync.dma_start(out=iota_sb[:], in_=iota_d[:])
        ident_sb = const.tile([128, 128], f32)
        nc.sync.dma_start(out=ident_sb[:], in_=ident_d[:])
        obbig16 = const.tile([128, NWIN * D], f16)
        obbig = const.tile([128, NWIN * D], mybir.dt.int8)
        scl_all = const.tile([128, NWIN], f32)
        mxall = const.tile([128, NWIN], f32)
        rqall = const.tile([128, NWIN], f32)

        start = 0
        for t in range(NWIN):
            c = c_list[t]
            acc = ps.tile([WIN, D], f32, tag="acc")
            for j in range(c):
                col = start + j
                # one offset per partition; dest [128,128] = one table row
                # per partition (the only indirect mode this walrus build
                # executes correctly).
                g = gp.tile([128, D], f16, tag="g")
                gi = nc.gpsimd.indirect_dma_start(
                    out=g[:],
                    out_offset=None,
                    in_=agout[:],
                    in_offset=bass.IndirectOffsetOnAxis(
                        ap=srci_sb[:, col : col + 1], axis=0
                    ),
                )
                # spread gathers over the 4 SWDGE queues for parallel
                # descriptor processing (indirect_dma_start pins queue 0)
                qn = col % 4
                if qn and _USE_QUEUES:
                    gi.ins.queue = f"qPoolDynamic{qn}"
                a_t = ap_pool.tile([128, WIN], f16, tag="A")
                nc.vector.tensor_scalar(
                    a_t[:],
                    iota_sb[:],
                    dst_sb[:, col : col + 1],
                    w_sb[:, col : col + 1],
                    mybir.AluOpType.is_equal,
                    mybir.AluOpType.mult,
                )
                nc.tensor.matmul(
                    out=acc[:],
                    lhsT=a_t[:],
                    rhs=g[:],
                    start=(j == 0),
                    stop=(j == c - 1),
                )
            nf = wk.tile([WIN, D], f16, tag="nf")
            nc.sync.dma_start(out=nf[:], in_=nfsh_d[t * WIN : (t + 1) * WIN, :])
            x = wk.tile([WIN, D], f32, tag="x")
            nc.vector.tensor_tensor(
                out=x[:], in0=nf[:], in1=acc[:], op=mybir.AluOpType.mult
            )
            xt_ps = ps.tile([D, WIN], f32, tag="xt")
            nc.tensor.transpose(out=xt_ps[:], in_=x[:], identity=ident_sb[:])
            xt = wk.tile([D, WIN], f16, tag="xts")
            nc.scalar.activation(
                out=xt[:], in_=xt_ps[:], func=mybir.ActivationFunctionType.Copy
            )
            op_ps = ps.tile([WIN, D], f32, tag="op")
            nc.tensor.matmul(
                out=op_ps[:], lhsT=xt[:], rhs=wt_sb[:], start=True, stop=True
            )
            nc.scalar.activation(
                out=obbig16[:, t * D : (t + 1) * D],
                in_=op_ps[:],
                func=mybir.ActivationFunctionType.Lrelu,
                alpha=0.01,
            )
            start += c
        # batch quantization: per-(p,t)-row absmax -> scale -> int8, in 4 ops
        ob3 = obbig16[:].rearrange("p (t d) -> p t d", d=D)
        nc.vector.tensor_reduce(
            out=mxall[:],
            in_=ob3,
            axis=mybir.AxisListType.X,
            op=mybir.AluOpType.max,
            apply_absolute_value=True,
        )
        # clamp away from 0 so all-zero rows give 0*huge = 0, not 0*inf=NaN
        nc.vector.tensor_scalar(
            scl_all[:],
            mxall[:],
            1.0 / 127.0,
            1e-30,
            mybir.AluOpType.mult,
            mybir.AluOpType.max,
        )
        nc.vector.reciprocal(out=rqall[:], in_=scl_all[:])
        rq3 = rqall[:].unsqueeze(2).broadcast_to([128, NWIN, D])
        nc.vector.tensor_tensor(
            out=obbig[:].rearrange("p (t d) -> p t d", d=D),
            in0=ob3,
            in1=rq3,
            op=mybir.AluOpType.mult,
        )
        nc.sync.dma_start(out=out_d[:, 0 : NWIN * D], in_=obbig[:])
        nc.sync.dma_start(
            out=out_d[:, NWIN * D :], in_=scl_all[:].bitcast(mybir.dt.int8)
        )
    _split_excess_waits(nc)
    return nc


def _get_exec(ct, c_list):
    key = (ct, tuple(c_list))
    if key in _cache:
        return _cache[key]

    import jax
    import jax.numpy as jnp
    from jax.sharding import Mesh, NamedSharding, PartitionSpec
    from jax.experimental.shard_map import shard_map
    from concourse.bass2jax import (
        _bass_exec_p,
        install_neuronx_cc_hook,
        partition_id_tensor,
    )

    install_neuronx_cc_hook()
    nc = _build_nc(ct, c_list)
    assert nc.dbg_addr is None
    partition_name = nc.partition_id_tensor.name if nc.partition_id_tensor else None

    in_names = []
    out_names = []
    out_avals = []
    for alloc in nc.m.functions[0].allocations:
        if not isinstance(alloc, mybir.MemoryLocationSet):
            continue
        name = alloc.memorylocations[0].name
        if alloc.kind == "ExternalInput":
            if name != partition_name:
                in_names.append(name)
        elif alloc.kind == "ExternalOutput":
            out_names.append(name)
            out_avals.append(
                jax.core.ShapedArray(
                    tuple(alloc.tensor_shape), mybir.dt.np(alloc.dtype)
                )
            )
    assert in_names == ["nfsh", "edgf"], in_names
    assert out_names == ["out"], out_names
    n_params = len(in_names)
    all_names = in_names + out_names
    if partition_name is not None:
        all_names.append(partition_name)
    all_names = tuple(all_names)

    def _body(*args):
        operands = list(args)
        if partition_name is not None:
            operands.append(partition_id_tensor())
        return tuple(
            _bass_exec_p.bind(
                *operands,
                out_avals=tuple(out_avals),
                in_names=all_names,
                out_names=tuple(out_names),
                lowering_input_output_aliases=(),
                sim_require_finite=True,
                sim_require_nnan=True,
                nc=nc,
            )
        )

    devices = jax.devices()[:N_CORES]
    mesh = Mesh(np.asarray(devices), ("core",))
    sh = NamedSharding(mesh, PartitionSpec("core"))
    n_outs = len(out_names)
    sharded = jax.jit(
        shard_map(
            _body,
            mesh=mesh,
            in_specs=(PartitionSpec("core"),) * (n_params + n_outs),
            out_specs=(PartitionSpec("core"),) * n_outs,
            check_rep=False,
        ),
        donate_argnums=tuple(range(n_params, n_params + n_outs)),
        keep_unused=True,
    )
    zeros_fn = jax.jit(
        lambda: jnp.zeros((N_CORES * 128, NWIN * D + NWIN * 4), jnp.int8),
        out_shardings=sh,
    )
    _cache[key] = (sharded, zeros_fn, sh)
    return _cache[key]


def _kernel_impl(nfeat, edge_src, edge_dst, edge_w, W):
    import jax

    n, d = nfeat.shape
    assert d == D and n == N_CORES * NPC

    # Stage the feature shards from a worker thread so the fp16 cast and the
    # transfer both overlap the edge sort below.
    import threading

    nfg_box = {}

    def _stage_nfg():
        nfg = np.zeros((GN, D), np.float16)
        nfg.reshape(N_CORES, PADN, D)[:, :NPC] = nfeat.reshape(N_CORES, NPC, D)
        if _cache:
            sh0 = next(iter(_cache.values()))[2]
            nfg_box["dev"] = jax.device_put(nfg, sh0)
        else:
            nfg_box["host"] = nfg

    nfg_thread = threading.Thread(target=_stage_nfg)
    nfg_thread.start()

    # Bucket edges by (dst core, 128-node window).  Sorting a uint16 window
    # key is ~2x faster than sorting the raw int32 dst.
    kd = edge_dst // NPC
    r = edge_dst - kd * NPC
    tw = r >> 7
    off16 = (r & 127).astype(np.float16)
    key = (kd * NWIN + tw).astype(np.uint16)
    order = np.argsort(key, kind="stable")
    key_s = key[order]
    ks = edge_src // NPC
    srcr_s = (edge_src + ks * (PADN - NPC)).astype(np.int32)[order]
    off_s = off16[order]
    w_s = edge_w[order].astype(np.float16)

    idx = np.searchsorted(key_s, np.arange(N_CORES * NWIN + 1))
    cnts = np.diff(idx).reshape(N_CORES, NWIN)

    c_list = [int(max(1, v)) for v in np.ceil(cnts / 128).max(axis=0).astype(int)]
    if sum(c_list) % 2:
        c_list[-1] += 1
    ct = int(sum(c_list))
    starts = np.concatenate([[0], np.cumsum(c_list)[:-1]]).astype(np.int32)

    # vectorized scatter into the padded chunk layout
    rnk = np.arange(key_s.size, dtype=np.int32) - idx[key_s].astype(np.int32)
    t_of = key_s % NWIN
    k_of = (key_s // NWIN).astype(np.int32)
    col = starts[t_of] + (rnk >> 7)
    row = rnk & 127

    sharded, zeros_fn, sh = _get_exec(ct, c_list)
    srci_g = np.zeros((N_CORES, 128, ct), np.int32)
    srci_g.ravel()[(k_of * 128 + row) * ct + col] = srcr_s

    edgf_g = np.zeros((N_CORES * 512, ct + 128), np.float16)
    e4 = edgf_g.reshape(N_CORES, 512, ct + 128)
    flat = (k_of * 512 + row) * (ct + 128) + col
    ev = edgf_g.ravel()
    ev[flat] = off_s
    ev[flat + 128 * (ct + 128)] = w_s
    src16 = srci_g.view(np.float16)  # [k, 128, 2*ct]
    e4[:, 256:384, 0:ct] = src16[:, :, 0:ct]
    e4[:, 384:512, 0:ct] = src16[:, :, ct:]
    wt16 = W.T.astype(np.float16)
    e4[:, 0:128, ct : ct + 128] = wt16
    edgf_dev = jax.device_put(edgf_g, sh)
    nfg_thread.join()
    nfg_dev = nfg_box.get("dev")
    if nfg_dev is None:
        nfg_dev = jax.device_put(nfg_box["host"], sh)

    (out_arr,) = sharded(nfg_dev, edgf_dev, zeros_fn())
    # device layout per core is [p, t*D+d] int8 plus f32 scales [p, t]
    # bitcast into the tail columns; output row r = t*128 + p
    res = np.empty((n, D), np.float32)

    def _dequant(k, shard):
        raw = np.asarray(shard)
        oi8 = raw[:, : NWIN * D].reshape(128, NWIN, D).transpose(1, 0, 2)
        scls = np.ascontiguousarray(raw[:, NWIN * D :]).view(np.float32)
        scls = scls.reshape(128, NWIN).T
        o = oi8.reshape(PADN, D)[:NPC].astype(np.float32)
        o *= scls.reshape(PADN, 1)[:NPC]
        res[k * NPC : (k + 1) * NPC] = o

    from concurrent.futures import ThreadPoolExecutor

    shards = out_arr.addressable_shards
    with ThreadPoolExecutor(4) as ex:
        list(ex.map(lambda ks: _dequant(ks[0], ks[1].data), enumerate(shards)))
    return res


def kernel(nfeat, edge_src, edge_dst, edge_w, W):
    return _kernel_impl(
        np.asarray(nfeat),
        np.asarray(edge_src),
        np.asarray(edge_dst),
        np.asarray(edge_w),
        np.asarray(W),
    )


# revision 42
# speedup vs baseline: 1.0291x; 1.0291x over previous
"""KGATConv GNN message-passing kernel for 8 Trainium2 NeuronCores.

Strategy (dst-node ownership; fp16 staging + on-device AllGather):
  - Core k owns nodes [k*12500, (k+1)*12500).  Host stages only core k's
    own feature rows (fp16, padded to 12544); the full gather table is
    rebuilt on-device with an HBM AllGather over NeuronLink -- host->device
    traffic for nfeat drops 16x vs replicating f32 to all cores.
  - Host sorts edges by dst and buckets per (core, 128-node window), padding
    each window's edge run to whole 128-edge chunks (chunk counts shared
    across cores so all 8 run one SPMD program).  Edge payload ships fp16
    (dst window offsets, weights, W^T) in one array + int32 src in another.
  - Device, per chunk: indirect-DMA gather of 128 fp16 rows (one offset per
    partition); DVE builds A[p,j] = w_p * (dst_p == j) in fp16; PE fp16
    matmul-accumulates h_nb = A^T @ msg in f32 PSUM.  Finalize per window:
    X = nfeat_own * h_nb, X^T via PE transpose, out = X @ W^T on PE (fp16),
    LeakyReLU on ACT, fp16 DMA out.
  - The PJRT executable, donated-zero outputs, and NEFF are all cached at
    module level so warm calls pay only staging + exec + fetch.
"""

import sys

sys.path.insert(0, "/opt/trn_rl_repo")

from contextlib import ExitStack

import numpy as np

import concourse.bass as bass
import concourse.mybir as mybir
import concourse.tile as tile

N_CORES = 8
D = 128
WIN = 128
NPC = 12500
NWIN = (NPC + WIN - 1) // WIN  # 98
PADN = NWIN * WIN  # 12544
GN = N_CORES * PADN  # 100352

_cache = {}
_USE_QUEUES = False


def _split_excess_waits(nc, maxw=1):
    # This walrus build rejects instructions carrying more than one sync
    # wait; move extras onto preceding single-wait NoOps on the same engine.
    for f in nc.m.functions:
        for bb in f.blocks:
            out = []
            for inst in bb.instructions:
                si = inst.sync_info
                waits = list(si.on_wait) if si and si.on_wait else []
                if len(waits) > maxw:
                    extra, keep = waits[:-maxw], waits[-maxw:]
                    for i in range(0, len(extra), maxw):
                        nop = mybir.InstNoOp(
                            name=nc.get_next_instruction_name(), ins=[], outs=[]
                        )
                        nop.engine = inst.engine
                        nop.sync_info = type(si)(
                            on_wait=extra[i : i + maxw], on_update=[]
                        )
                        nc.register_instruction(nop, overwrite=True)
                        out.append(nop)
                    si.on_wait = keep
                out.append(inst)
            bb.instructions[:] = out


def _build_nc(ct, c_list):
    f16 = mybir.dt.float16
    f32 = mybir.dt.float32
    nc = bass.Bass(num_swdge_queues=4)
    assert ct % 2 == 0
    nfsh_d = nc.declare_dram_parameter("nfsh", [PADN, D], f16, isOutput=False)
    # rows 0:128 dst-offset f16, 128:256 w f16, 256:512 the int32 src indices
    # bitcast to f16 (two half-row blocks); W^T at [0:128, ct:ct+128].
    # One param = one host device_put (~100ms saved vs a separate int32 put).
    edgf_d = nc.declare_dram_parameter("edgf", [512, ct + 128], f16, isOutput=False)
    # int8 output with a per-row f32 scale (row absmax / 127) halves the
    # device->host fetch; quantization error is <= 1/127 of the row max.
    # Partition-major layout [p, t*D+d] so the whole thing ships as one DMA
    # (host untransposes).  The f32 scales ride along bitcast into the last
    # NWIN*4 int8 columns: a second ExternalOutput costs ~100ms of dispatch.
    out_d = nc.declare_dram_parameter(
        "out", [128, NWIN * D + NWIN * 4], mybir.dt.int8, isOutput=True
    )
    iota_d = nc.inline_tensor(
        np.tile(np.arange(WIN, dtype=np.float32), (128, 1)), name="iota"
    )
    ident_d = nc.inline_tensor(np.eye(128, dtype=np.float32), name="ident")

    with tile.TileContext(nc) as tc, ExitStack() as ctx:
        dram = ctx.enter_context(tc.tile_pool(name="dram", bufs=1, space="DRAM"))
        agin = dram.tile([PADN, D], f16)
        agout = dram.tile([GN, D], f16, addr_space="Shared")
        const = ctx.enter_context(tc.tile_pool(name="const", bufs=1))
        gp = ctx.enter_context(tc.tile_pool(name="gp", bufs=10))
        ap_pool = ctx.enter_context(tc.tile_pool(name="ap", bufs=4))
        wk = ctx.enter_context(tc.tile_pool(name="wk", bufs=3))
        ps = ctx.enter_context(tc.tile_pool(name="ps", bufs=2, space="PSUM"))

        # kick off the gather-table rebuild first; const loads overlap it
        nc.sync.dma_start(out=agin[:], in_=nfsh_d[:])
        nc.gpsimd.collective_compute(
            "AllGather",
            mybir.AluOpType.bypass,
            replica_groups=[list(range(N_CORES))],
            ins=[agin.opt()],
            outs=[agout.opt()],
        )

        srci_sb = const.tile([128, ct], mybir.dt.int32)
        nc.sync.dma_start(
            out=srci_sb[:, 0 : ct // 2].bitcast(f16), in_=edgf_d[256:384, 0:ct]
        )
        nc.sync.dma_start(
            out=srci_sb[:, ct // 2 : ct].bitcast(f16), in_=edgf_d[384:512, 0:ct]
        )
        dst_sb16 = const.tile([128, ct], f16)
        nc.sync.dma_start(out=dst_sb16[:], in_=edgf_d[0:128, 0:ct])
        w_sb16 = const.tile([128, ct], f16)
        nc.sync.dma_start(out=w_sb16[:], in_=edgf_d[128:256, 0:ct])
        # DVE scalar operands must be f32; widen once on device
        dst_sb = const.tile([128, ct], f32)
        nc.scalar.activation(
            out=dst_sb[:], in_=dst_sb16[:], func=mybir.ActivationFunctionType.Copy
        )
        w_sb = const.tile([128, ct], f32)
        nc.scalar.activation(
            out=w_sb[:], in_=w_sb16[:], func=mybir.ActivationFunctionType.Copy
        )
        wt_sb = const.tile([128, 128], f16)
        nc.sync.dma_start(out=wt_sb[:], in_=edgf_d[0:128, ct : ct + 128])
        iota_sb = const.tile([128, WIN], f32)
        nc.sync.dma_start(out=iota_sb[:], in_=iota_d[:])
        ident_sb = const.tile([128, 128], f32)
        nc.sync.dma_start(out=ident_sb[:], in_=ident_d[:])
        obbig16 = const.tile([128, NWIN * D], f16)
        obbig = const.tile([128, NWIN * D], mybir.dt.int8)
        scl_all = const.tile([128, NWIN], f32)
        mxall = const.tile([128, NWIN], f32)
        rqall = const.tile([128, NWIN], f32)

        start = 0
        for t in range(NWIN):
            c = c_list[t]
            acc = ps.tile([WIN, D], f32, tag="acc")
            for j in range(c):
                col = start + j
                # one offset per partition; dest [128,128] = one table row
                # per partition (the only indirect mode this walrus build
                # executes correctly).
                g = gp.tile([128, D], f16, tag="g")
                gi = nc.gpsimd.indirect_dma_start(
                    out=g[:],
                    out_offset=None,
                    in_=agout[:],
                    in_offset=bass.IndirectOffsetOnAxis(
                        ap=srci_sb[:, col : col + 1], axis=0
                    ),
                )
                # spread gathers over the 4 SWDGE queues for parallel
                # descriptor processing (indirect_dma_start pins queue 0)
                qn = col % 4
                if qn and _USE_QUEUES:
                    gi.ins.queue = f"qPoolDynamic{qn}"
                a_t = ap_pool.tile([128, WIN], f16, tag="A")
                nc.vector.tensor_scalar(
                    a_t[:],
                    iota_sb[:],
                    dst_sb[:, col : col + 1],
                    w_sb[:, col : col + 1],
                    mybir.AluOpType.is_equal,
                    mybir.AluOpType.mult,
                )
                nc.tensor.matmul(
                    out=acc[:],
                    lhsT=a_t[:],
                    rhs=g[:],
                    start=(j == 0),
                    stop=(j == c - 1),
                )
            nf = wk.tile([WIN, D], f16, tag="nf")
            nc.sync.dma_start(out=nf[:], in_=nfsh_d[t * WIN : (t + 1) * WIN, :])
            x = wk.tile([WIN, D], f32, tag="x")
            nc.vector.tensor_tensor(
                out=x[:], in0=nf[:], in1=acc[:], op=mybir.AluOpType.mult
            )
            xt_ps = ps.tile([D, WIN], f32, tag="xt")
            nc.tensor.transpose(out=xt_ps[:], in_=x[:], identity=ident_sb[:])
            xt = wk.tile([D, WIN], f16, tag="xts")
            nc.scalar.activation(
                out=xt[:], in_=xt_ps[:], func=mybir.ActivationFunctionType.Copy
            )
            op_ps = ps.tile([WIN, D], f32, tag="op")
            nc.tensor.matmul(
                out=op_ps[:], lhsT=xt[:], rhs=wt_sb[:], start=True, stop=True
            )
            nc.scalar.activation(
                out=obbig16[:, t * D : (t + 1) * D],
                in_=op_ps[:],
                func=mybir.ActivationFunctionType.Lrelu,
                alpha=0.01,
            )
            start += c
        # batch quantization: per-(p,t)-row absmax -> scale -> int8, in 4 ops
        ob3 = obbig16[:].rearrange("p (t d) -> p t d", d=D)
        nc.vector.tensor_reduce(
            out=mxall[:],
            in_=ob3,
            axis=mybir.AxisListType.X,
            op=mybir.AluOpType.max,
            apply_absolute_value=True,
        )
        # clamp away from 0 so all-zero rows give 0*huge = 0, not 0*inf=NaN
        nc.vector.tensor_scalar(
            scl_all[:],
            mxall[:],
            1.0 / 127.0,
            1e-30,
            mybir.AluOpType.mult,
            mybir.AluOpType.max,
        )
        nc.vector.reciprocal(out=rqall[:], in_=scl_all[:])
        rq3 = rqall[:].unsqueeze(2).broadcast_to([128, NWIN, D])
        nc.vector.tensor_tensor(
            out=obbig[:].rearrange("p (t d) -> p t d", d=D),
            in0=ob3,
            in1=rq3,
            op=mybir.AluOpType.mult,
        )
        nc.sync.dma_start(out=out_d[:, 0 : NWIN * D], in_=obbig[:])
        nc.sync.dma_start(
            out=out_d[:, NWIN * D :], in_=scl_all[:].bitcast(mybir.dt.int8)
        )
    _split_excess_waits(nc)
    return nc


def _get_exec(ct, c_list):
    key = (ct, tuple(c_list))
    if key in _cache:
        return _cache[key]

    import jax
    import jax.numpy as jnp
    from jax.sharding import Mesh, NamedSharding, PartitionSpec
    from jax.experimental.shard_map import shard_map
    from concourse.bass2jax import (
        _bass_exec_p,
        install_neuronx_cc_hook,
        partition_id_tensor,
    )

    install_neuronx_cc_hook()
    nc = _build_nc(ct, c_list)
    assert nc.dbg_addr is None
    partition_name = nc.partition_id_tensor.name if nc.partition_id_tensor else None

    in_names = []
    out_names = []
    out_avals = []
    for alloc in nc.m.functions[0].allocations:
        if not isinstance(alloc, mybir.MemoryLocationSet):
            continue
        name = alloc.memorylocations[0].name
        if alloc.kind == "ExternalInput":
            if name != partition_name:
                in_names.append(name)
        elif alloc.kind == "ExternalOutput":
            out_names.append(name)
            out_avals.append(
                jax.core.ShapedArray(
                    tuple(alloc.tensor_shape), mybir.dt.np(alloc.dtype)
                )
            )
    assert in_names == ["nfsh", "edgf"], in_names
    assert out_names == ["out"], out_names
    n_params = len(in_names)
    all_names = in_names + out_names
    if partition_name is not None:
        all_names.append(partition_name)
    all_names = tuple(all_names)

    def _body(*args):
        operands = list(args)
        if partition_name is not None:
            operands.append(partition_id_tensor())
        return tuple(
            _bass_exec_p.bind(
                *operands,
                out_avals=tuple(out_avals),
                in_names=all_names,
                out_names=tuple(out_names),
                lowering_input_output_aliases=(),
                sim_require_finite=True,
                sim_require_nnan=True,
                nc=nc,
            )
        )

    devices = jax.devices()[:N_CORES]
    mesh = Mesh(np.asarray(devices), ("core",))
    sh = NamedSharding(mesh, PartitionSpec("core"))
    n_outs = len(out_names)
    sharded = jax.jit(
        shard_map(
            _body,
            mesh=mesh,
            in_specs=(PartitionSpec("core"),) * (n_params + n_outs),
            out_specs=(PartitionSpec("core"),) * n_outs,
            check_rep=False,
        ),
        donate_argnums=tuple(range(n_params, n_params + n_outs)),
        keep_unused=True,
    )
    zeros_fn = jax.jit(
        lambda: jnp.zeros((N_CORES * 128, NWIN * D + NWIN * 4), jnp.int8),
        out_shardings=sh,
    )
    _cache[key] = (sharded, zeros_fn, sh)
    return _cache[key]


def _kernel_impl(nfeat, edge_src, edge_dst, edge_w, W):
    import jax

    n, d = nfeat.shape
    assert d == D and n == N_CORES * NPC

    # Core k owns node rows [k*12544, (k+1)*12544): PADN-aligned ownership
    # makes the window key a shift, the offset a mask, and the gather table
    # index the raw node id (no remap).  Stage features first so the
    # transfer overlaps the edge sort (device_put is async).
    nfg = np.zeros((GN, D), np.float16)
    nfg[:n] = nfeat
    nfg_dev = None
    if _cache:
        sh0 = next(iter(_cache.values()))[2]
        nfg_dev = jax.device_put(nfg, sh0)

    key = (edge_dst >> 7).astype(np.uint16)
    off16 = (edge_dst & 127).astype(np.float16)
    order = np.argsort(key, kind="stable")
    key_s = key[order]
    srcr_s = edge_src[order]
    off_s = off16[order]
    w_s = edge_w[order].astype(np.float16)

    idx = np.searchsorted(key_s, np.arange(N_CORES * NWIN + 1))
    cnts = np.diff(idx).reshape(N_CORES, NWIN)

    c_list = [int(max(1, v)) for v in np.ceil(cnts / 128).max(axis=0).astype(int)]
    if sum(c_list) % 2:
        c_list[-1] += 1
    ct = int(sum(c_list))
    starts = np.concatenate([[0], np.cumsum(c_list)[:-1]]).astype(np.int32)

    # vectorized scatter into the padded chunk layout
    rnk = np.arange(key_s.size, dtype=np.int32) - idx[key_s].astype(np.int32)
    t_of = key_s % NWIN
    k_of = (key_s // NWIN).astype(np.int32)
    col = starts[t_of] + (rnk >> 7)
    row = rnk & 127

    sharded, zeros_fn, sh = _get_exec(ct, c_list)
    srci_g = np.zeros((N_CORES, 128, ct), np.int32)
    srci_g.ravel()[(k_of * 128 + row) * ct + col] = srcr_s

    edgf_g = np.zeros((N_CORES * 512, ct + 128), np.float16)
    e4 = edgf_g.reshape(N_CORES, 512, ct + 128)
    flat = (k_of * 512 + row) * (ct + 128) + col
    ev = edgf_g.ravel()
    ev[flat] = off_s
    ev[flat + 128 * (ct + 128)] = w_s
    src16 = srci_g.view(np.float16)  # [k, 128, 2*ct]
    e4[:, 256:384, 0:ct] = src16[:, :, 0:ct]
    e4[:, 384:512, 0:ct] = src16[:, :, ct:]
    wt16 = W.T.astype(np.float16)
    e4[:, 0:128, ct : ct + 128] = wt16
    edgf_dev = jax.device_put(edgf_g, sh)
    if nfg_dev is None:
        nfg_dev = jax.device_put(nfg, sh)

    (out_arr,) = sharded(nfg_dev, edgf_dev, zeros_fn())
    # device layout per core is [p, t*D+d] int8 plus f32 scales [p, t]
    # bitcast into the tail columns; output row r = t*128 + p
    res = np.empty((n, D), np.float32)

    def _dequant(k, shard):
        nk = min(PADN, n - k * PADN)
        if nk <= 0:
            return
        raw = np.asarray(shard)
        oi8 = raw[:, : NWIN * D].reshape(128, NWIN, D).transpose(1, 0, 2)
        scls = np.ascontiguousarray(raw[:, NWIN * D :]).view(np.float32)
        scls = scls.reshape(128, NWIN).T
        o = oi8.reshape(PADN, D)[:nk].astype(np.float32)
        o *= scls.reshape(PADN, 1)[:nk]
        res[k * PADN : k * PADN + nk] = o

    from concurrent.futures import ThreadPoolExecutor

    shards = out_arr.addressable_shards
    with ThreadPoolExecutor(4) as ex:
        list(ex.map(lambda ks: _dequant(ks[0], ks[1].data), enumerate(shards)))
    return res


def kernel(nfeat, edge_src, edge_dst, edge_w, W):
    return _kernel_impl(
        np.asarray(nfeat),
        np.asarray(edge_src),
        np.asarray(edge_dst),
        np.asarray(edge_w),
        np.asarray(W),
    )


# revision 44
# speedup vs baseline: 1.0601x; 1.0302x over previous
"""KGATConv GNN message-passing kernel for 8 Trainium2 NeuronCores.

Strategy (dst-node ownership; fp16 staging + on-device AllGather):
  - Core k owns nodes [k*12500, (k+1)*12500).  Host stages only core k's
    own feature rows (fp16, padded to 12544); the full gather table is
    rebuilt on-device with an HBM AllGather over NeuronLink -- host->device
    traffic for nfeat drops 16x vs replicating f32 to all cores.
  - Host sorts edges by dst and buckets per (core, 128-node window), padding
    each window's edge run to whole 128-edge chunks (chunk counts shared
    across cores so all 8 run one SPMD program).  Edge payload ships fp16
    (dst window offsets, weights, W^T) in one array + int32 src in another.
  - Device, per chunk: indirect-DMA gather of 128 fp16 rows (one offset per
    partition); DVE builds A[p,j] = w_p * (dst_p == j) in fp16; PE fp16
    matmul-accumulates h_nb = A^T @ msg in f32 PSUM.  Finalize per window:
    X = nfeat_own * h_nb, X^T via PE transpose, out = X @ W^T on PE (fp16),
    LeakyReLU on ACT, fp16 DMA out.
  - The PJRT executable, donated-zero outputs, and NEFF are all cached at
    module level so warm calls pay only staging + exec + fetch.
"""

import sys

sys.path.insert(0, "/opt/trn_rl_repo")

from contextlib import ExitStack

import numpy as np

import concourse.bass as bass
import concourse.mybir as mybir
import concourse.tile as tile

N_CORES = 8
D = 128
WIN = 128
NPC = 12500
NWIN = (NPC + WIN - 1) // WIN  # 98
PADN = NWIN * WIN  # 12544
GN = N_CORES * PADN  # 100352

_cache = {}
_USE_QUEUES = False


def _split_excess_waits(nc, maxw=1):
    # This walrus build rejects instructions carrying more than one sync
    # wait; move extras onto preceding single-wait NoOps on the same engine.
    for f in nc.m.functions:
        for bb in f.blocks:
            out = []
            for inst in bb.instructions:
                si = inst.sync_info
                waits = list(si.on_wait) if si and si.on_wait else []
                if len(waits) > maxw:
                    extra, keep = waits[:-maxw], waits[-maxw:]
                    for i in range(0, len(extra), maxw):
                        nop = mybir.InstNoOp(
                            name=nc.get_next_instruction_name(), ins=[], outs=[]
                        )
                        nop.engine = inst.engine
                        nop.sync_info = type(si)(
                            on_wait=extra[i : i + maxw], on_update=[]
                        )
                        nc.register_instruction(nop, overwrite=True)
                        out.append(nop)
                    si.on_wait = keep
                out.append(inst)
            bb.instructions[:] = out


def _build_nc(ct, c_list):
    f16 = mybir.dt.float16
    f32 = mybir.dt.float32
    nc = bass.Bass(num_swdge_queues=4)
    assert ct % 2 == 0
    nfsh_d = nc.declare_dram_parameter("nfsh", [PADN, D], f16, isOutput=False)
    # rows 0:128 dst-offset f16, 128:256 w f16, 256:512 the int32 src indices
    # bitcast to f16 (two half-row blocks); W^T at [0:128, ct:ct+128].
    # One param = one host device_put (~100ms saved vs a separate int32 put).
    edgf_d = nc.declare_dram_parameter("edgf", [512, ct + 128], f16, isOutput=False)
    # int8 output with a per-row f32 scale (row absmax / 127) halves the
    # device->host fetch; quantization error is <= 1/127 of the row max.
    # Partition-major layout [p, t*D+d] so the whole thing ships as one DMA
    # (host untransposes).  The f32 scales ride along bitcast into the last
    # NWIN*4 int8 columns: a second ExternalOutput costs ~100ms of dispatch.
    out_d = nc.declare_dram_parameter(
        "out", [128, NWIN * D + NWIN * 4], mybir.dt.int8, isOutput=True
    )
    iota_d = nc.inline_tensor(
        np.tile(np.arange(WIN, dtype=np.float32), (128, 1)), name="iota"
    )
    ident_d = nc.inline_tensor(np.eye(128, dtype=np.float32), name="ident")

    with tile.TileContext(nc) as tc, ExitStack() as ctx:
        dram = ctx.enter_context(tc.tile_pool(name="dram", bufs=1, space="DRAM"))
        agin = dram.tile([PADN, D], f16)
        agout = dram.tile([GN, D], f16, addr_space="Shared")
        const = ctx.enter_context(tc.tile_pool(name="const", bufs=1))
        gp = ctx.enter_context(tc.tile_pool(name="gp", bufs=10))
        ap_pool = ctx.enter_context(tc.tile_pool(name="ap", bufs=4))
        wk = ctx.enter_context(tc.tile_pool(name="wk", bufs=3))
        ps = ctx.enter_context(tc.tile_pool(name="ps", bufs=2, space="PSUM"))

        # kick off the gather-table rebuild first; const loads overlap it
        nc.sync.dma_start(out=agin[:], in_=nfsh_d[:])
        nc.gpsimd.collective_compute(
            "AllGather",
            mybir.AluOpType.bypass,
            replica_groups=[list(range(N_CORES))],
            ins=[agin.opt()],
            outs=[agout.opt()],
        )

        srci_sb = const.tile([128, ct], mybir.dt.int32)
        nc.sync.dma_start(
            out=srci_sb[:, 0 : ct // 2].bitcast(f16), in_=edgf_d[256:384, 0:ct]
        )
        nc.sync.dma_start(
            out=srci_sb[:, ct // 2 : ct].bitcast(f16), in_=edgf_d[384:512, 0:ct]
        )
        dst_sb16 = const.tile([128, ct], f16)
        nc.sync.dma_start(out=dst_sb16[:], in_=edgf_d[0:128, 0:ct])
        w_sb16 = const.tile([128, ct], f16)
        nc.sync.dma_start(out=w_sb16[:], in_=edgf_d[128:256, 0:ct])
        # DVE scalar operands must be f32; widen once on device
        dst_sb = const.tile([128, ct], f32)
        nc.scalar.activation(
            out=dst_sb[:], in_=dst_sb16[:], func=mybir.ActivationFunctionType.Copy
        )
        w_sb = const.tile([128, ct], f32)
        nc.scalar.activation(
            out=w_sb[:], in_=w_sb16[:], func=mybir.ActivationFunctionType.Copy
        )
        wt_sb = const.tile([128, 128], f16)
        nc.sync.dma_start(out=wt_sb[:], in_=edgf_d[0:128, ct : ct + 128])
        iota_sb = const.tile([128, WIN], f32)
        nc.sync.dma_start(out=iota_sb[:], in_=iota_d[:])
        ident_sb = const.tile([128, 128], f32)
        nc.sync.dma_start(out=ident_sb[:], in_=ident_d[:])
        obbig16 = const.tile([128, NWIN * D], f16)
        obbig = const.tile([128, NWIN * D], mybir.dt.int8)
        scl_all = const.tile([128, NWIN], f32)
        mxall = const.tile([128, NWIN], f32)
        rqall = const.tile([128, NWIN], f32)

        start = 0
        for t in range(NWIN):
            c = c_list[t]
            acc = ps.tile([WIN, D], f32, tag="acc")
            for j in range(c):
                col = start + j
                # one offset per partition; dest [128,128] = one table row
                # per partition (the only indirect mode this walrus build
                # executes correctly).
                g = gp.tile([128, D], f16, tag="g")
                gi = nc.gpsimd.indirect_dma_start(
                    out=g[:],
                    out_offset=None,
                    in_=agout[:],
                    in_offset=bass.IndirectOffsetOnAxis(
                        ap=srci_sb[:, col : col + 1], axis=0
                    ),
                )
                # spread gathers over the 4 SWDGE queues for parallel
                # descriptor processing (indirect_dma_start pins queue 0)
                qn = col % 4
                if qn and _USE_QUEUES:
                    gi.ins.queue = f"qPoolDynamic{qn}"
                a_t = ap_pool.tile([128, WIN], f16, tag="A")
                nc.vector.tensor_scalar(
                    a_t[:],
                    iota_sb[:],
                    dst_sb[:, col : col + 1],
                    w_sb[:, col : col + 1],
                    mybir.AluOpType.is_equal,
                    mybir.AluOpType.mult,
                )
                nc.tensor.matmul(
                    out=acc[:],
                    lhsT=a_t[:],
                    rhs=g[:],
                    start=(j == 0),
                    stop=(j == c - 1),
                )
            nf = wk.tile([WIN, D], f16, tag="nf")
            nc.sync.dma_start(out=nf[:], in_=nfsh_d[t * WIN : (t + 1) * WIN, :])
            x = wk.tile([WIN, D], f32, tag="x")
            nc.vector.tensor_tensor(
                out=x[:], in0=nf[:], in1=acc[:], op=mybir.AluOpType.mult
            )
            xt_ps = ps.tile([D, WIN], f32, tag="xt")
            nc.tensor.transpose(out=xt_ps[:], in_=x[:], identity=ident_sb[:])
            xt = wk.tile([D, WIN], f16, tag="xts")
            nc.scalar.activation(
                out=xt[:], in_=xt_ps[:], func=mybir.ActivationFunctionType.Copy
            )
            op_ps = ps.tile([WIN, D], f32, tag="op")
            nc.tensor.matmul(
                out=op_ps[:], lhsT=xt[:], rhs=wt_sb[:], start=True, stop=True
            )
            nc.scalar.activation(
                out=obbig16[:, t * D : (t + 1) * D],
                in_=op_ps[:],
                func=mybir.ActivationFunctionType.Lrelu,
                alpha=0.01,
            )
            start += c
        # batch quantization: per-(p,t)-row absmax -> scale -> int8, in 4 ops
        ob3 = obbig16[:].rearrange("p (t d) -> p t d", d=D)
        nc.vector.tensor_reduce(
            out=mxall[:],
            in_=ob3,
            axis=mybir.AxisListType.X,
            op=mybir.AluOpType.max,
            apply_absolute_value=True,
        )
        # clamp away from 0 so all-zero rows give 0*huge = 0, not 0*inf=NaN
        nc.vector.tensor_scalar(
            scl_all[:],
            mxall[:],
            1.0 / 127.0,
            1e-30,
            mybir.AluOpType.mult,
            mybir.AluOpType.max,
        )
        nc.vector.reciprocal(out=rqall[:], in_=scl_all[:])
        rq3 = rqall[:].unsqueeze(2).broadcast_to([128, NWIN, D])
        nc.vector.tensor_tensor(
            out=obbig[:].rearrange("p (t d) -> p t d", d=D),
            in0=ob3,
            in1=rq3,
            op=mybir.AluOpType.mult,
        )
        nc.sync.dma_start(out=out_d[:, 0 : NWIN * D], in_=obbig[:])
        nc.sync.dma_start(
            out=out_d[:, NWIN * D :], in_=scl_all[:].bitcast(mybir.dt.int8)
        )
    _split_excess_waits(nc)
    return nc


def _get_exec(ct, c_list):
    key = (ct, tuple(c_list))
    if key in _cache:
        return _cache[key]

    import jax
    import jax.numpy as jnp
    from jax.sharding import Mesh, NamedSharding, PartitionSpec
    from jax.experimental.shard_map import shard_map
    from concourse.bass2jax import (
        _bass_exec_p,
        install_neuronx_cc_hook,
        partition_id_tensor,
    )

    install_neuronx_cc_hook()
    nc = _build_nc(ct, c_list)
    assert nc.dbg_addr is None
    partition_name = nc.partition_id_tensor.name if nc.partition_id_tensor else None

    in_names = []
    out_names = []
    out_avals = []
    for alloc in nc.m.functions[0].allocations:
        if not isinstance(alloc, mybir.MemoryLocationSet):
            continue
        name = alloc.memorylocations[0].name
        if alloc.kind == "ExternalInput":
            if name != partition_name:
                in_names.append(name)
        elif alloc.kind == "ExternalOutput":
            out_names.append(name)
            out_avals.append(
                jax.core.ShapedArray(
                    tuple(alloc.tensor_shape), mybir.dt.np(alloc.dtype)
                )
            )
    assert in_names == ["nfsh", "edgf"], in_names
    assert out_names == ["out"], out_names
    n_params = len(in_names)
    all_names = in_names + out_names
    if partition_name is not None:
        all_names.append(partition_name)
    all_names = tuple(all_names)

    def _body(*args):
        operands = list(args)
        if partition_name is not None:
            operands.append(partition_id_tensor())
        return tuple(
            _bass_exec_p.bind(
                *operands,
                out_avals=tuple(out_avals),
                in_names=all_names,
                out_names=tuple(out_names),
                lowering_input_output_aliases=(),
                sim_require_finite=True,
                sim_require_nnan=True,
                nc=nc,
            )
        )

    devices = jax.devices()[:N_CORES]
    mesh = Mesh(np.asarray(devices), ("core",))
    sh = NamedSharding(mesh, PartitionSpec("core"))
    n_outs = len(out_names)
    sharded = jax.jit(
        shard_map(
            _body,
            mesh=mesh,
            in_specs=(PartitionSpec("core"),) * (n_params + n_outs),
            out_specs=(PartitionSpec("core"),) * n_outs,
            check_rep=False,
        ),
        donate_argnums=tuple(range(n_params, n_params + n_outs)),
        keep_unused=True,
    )
    zeros_fn = jax.jit(
        lambda: jnp.zeros((N_CORES * 128, NWIN * D + NWIN * 4), jnp.int8),
        out_shardings=sh,
    )
    _cache[key] = (sharded, zeros_fn, sh)
    return _cache[key]


def _kernel_impl(nfeat, edge_src, edge_dst, edge_w, W):
    import jax

    n, d = nfeat.shape
    assert d == D and n == N_CORES * NPC

    # Core k owns node rows [k*12544, (k+1)*12544): PADN-aligned ownership
    # makes the window key a shift, the offset a mask, and the gather table
    # index the raw node id (no remap).  Stage features first so the
    # transfer overlaps the edge sort (device_put is async).
    nfg = np.zeros((GN, D), np.float16)
    nfg[:n] = nfeat
    nfg_dev = None
    if _cache:
        sh0 = next(iter(_cache.values()))[2]
        nfg_dev = jax.device_put(nfg, sh0)

    key = (edge_dst >> 7).astype(np.uint16)
    off16 = (edge_dst & 127).astype(np.float16)
    order = np.argsort(key, kind="stable")
    key_s = key[order]
    srcr_s = edge_src[order]
    off_s = off16[order]
    w_s = edge_w[order].astype(np.float16)

    idx = np.searchsorted(key_s, np.arange(N_CORES * NWIN + 1))
    cnts = np.diff(idx).reshape(N_CORES, NWIN)

    c_list = [int(max(1, v)) for v in np.ceil(cnts / 128).max(axis=0).astype(int)]
    if sum(c_list) % 2:
        c_list[-1] += 1
    ct = int(sum(c_list))
    starts = np.concatenate([[0], np.cumsum(c_list)[:-1]]).astype(np.int32)

    # vectorized scatter into the padded chunk layout
    rnk = np.arange(key_s.size, dtype=np.int32) - idx[key_s].astype(np.int32)
    t_of = key_s % NWIN
    k_of = (key_s // NWIN).astype(np.int32)
    col = starts[t_of] + (rnk >> 7)
    row = rnk & 127

    sharded, zeros_fn, sh = _get_exec(ct, c_list)
    srci_g = np.zeros((N_CORES, 128, ct), np.int32)
    srci_g.ravel()[(k_of * 128 + row) * ct + col] = srcr_s

    edgf_g = np.zeros((N_CORES * 512, ct + 128), np.float16)
    e4 = edgf_g.reshape(N_CORES, 512, ct + 128)
    flat = (k_of * 512 + row) * (ct + 128) + col
    ev = edgf_g.ravel()
    ev[flat] = off_s
    ev[flat + 128 * (ct + 128)] = w_s
    src16 = srci_g.view(np.float16)  # [k, 128, 2*ct]
    e4[:, 256:384, 0:ct] = src16[:, :, 0:ct]
    e4[:, 384:512, 0:ct] = src16[:, :, ct:]
    wt16 = W.T.astype(np.float16)
    e4[:, 0:128, ct : ct + 128] = wt16
    edgf_dev = jax.device_put(edgf_g, sh)
    if nfg_dev is None:
        nfg_dev = jax.device_put(nfg, sh)

    (out_arr,) = sharded(nfg_dev, edgf_dev, zeros_fn())
    # device layout per core is [p, t*D+d] int8 plus f32 scales [p, t]
    # bitcast into the tail columns; output row r = t*128 + p
    res = np.empty((n, D), np.float32)

    def _dequant(k, shard):
        nk = min(PADN, n - k * PADN)
        if nk <= 0:
            return
        raw = np.asarray(shard)
        oi8 = raw[:, : NWIN * D].reshape(128, NWIN, D).transpose(1, 0, 2)
        scls = np.ascontiguousarray(raw[:, NWIN * D :]).view(np.float32)
        scls = scls.reshape(128, NWIN).T
        o = oi8.reshape(PADN, D)[:nk].astype(np.float32)
        o *= scls.reshape(PADN, 1)[:nk]
        res[k * PADN : k * PADN + nk] = o

    from concurrent.futures import ThreadPoolExecutor

    shards = out_arr.addressable_shards
    with ThreadPoolExecutor(4) as ex:
        list(ex.map(lambda ks: _dequant(ks[0], ks[1].data), enumerate(shards)))
    return res


def kernel(nfeat, edge_src, edge_dst, edge_w, W):
    return _kernel_impl(
        np.asarray(nfeat),
        np.asarray(edge_src),
        np.asarray(edge_dst),
        np.asarray(edge_w),
        np.asarray(W),
    )
